# revision 7
# baseline (speedup 1.0000x reference)
"""LoCon1d (position-specific conv1d) Trainium2 kernel, v3.

out[b,o,s] = sum_{c,k} xpad[b,c,s+k] * w[o,c,s,k] + bias[o,s]
x (16,64,1024) f32, w (64,64,1024,3) f32, bias (64,1024) f32.

Sequence-parallel over 8 cores (128 positions each). Per core the 128
positions form 64 "pairs" (j, 64+j) packed block-diagonally into the
128-row contraction dim of the PE (rows 0:64 = Cin half A, rows 64:128
= Cin half B); stationary lhsT = x window [128, 32] (cols = 2 halves x
16 batch), moving rhs = weight columns (Cout).

x ships dense: xd [128, 16, TW] f16 (half A on partitions 0:64, half B
on 64:128) -> two 64-partition DMAs into the diagonal blocks of the
SBUF tile xr [128, 32, TW]; the off-diagonal blocks are DVE-memset to
zero. lhsT for window t is the strided slice xr[:, :, t].

Weights are laid out host-side in exact PE consumption order
[128, 12288] f16: 16 compute chunks of 4 pairs x (6 windows x up-to-3
positions x 64 Cout), DMA'd as 8 transfers of 2 chunks. Window t
serves positions t-2..t (tap k = t-p); consecutive windows write
overlapping, shifted psum ranges; per-element PSUM has_written handles
the staggered accumulation.

Col-tiling: compute chunk cc -> col-group g=cc%4 -> psum partitions
32g:32g+32; one [128, 256] psum bank covers 4 chunks (16 pairs); 4
banks cover the core. Bias is pre-added per bank by a K=8 indicator
matmul (start=True writes the whole bank). DVE evacuates psum -> f16
sbuf tiles, DMA'd out as [128, 256].
"""

import numpy as np

import concourse.bass as bass
import concourse.mybir as mybir
import concourse.tile as tile
from concourse import bacc, bass_utils

N_CORES = 8
B, CIN, COUT, S, K = 16, 64, 64, 1024, 3
SC = S // N_CORES          # positions per core (128)
H = SC // 2                # pairs per core (64)
TW = H + K - 1             # x window count (66)
NCC = 16                   # compute chunks
PPC = H // NCC             # pairs per chunk (4)
NWIN = PPC + K - 1         # windows per chunk (6)
NGRP = 4                   # psum col-groups
NBANK = NCC // NGRP        # psum banks (4)
BANKC = PPC * 64           # psum cols per bank-group (256)
WCOLS = 64                 # moving cols per (window, position) = Cout
NDMA = 8                   # weight DMA transfers
CCPD = NCC // NDMA         # compute chunks per DMA (2)

# per-window position counts within a chunk and column offsets
_WIDTHS = [min(jj, K - 1, PPC - 1 + K - 1 - jj) + 1 for jj in range(NWIN)]
# widths = [1,2,3,3,2,1]
_OFFS = np.concatenate([[0], np.cumsum(_WIDTHS)]).tolist()
CHUNK_COLS = _OFFS[-1] * WCOLS      # 768
TOT_COLS = CHUNK_COLS * NCC         # 12288

_DT = {"f32": mybir.dt.float32, "bf16": mybir.dt.bfloat16,
       "f16": mybir.dt.float16}

DTYPE = "f16"


def _np_dt(dt):
    if dt == "bf16":
        import ml_dtypes
        return ml_dtypes.bfloat16
    if dt == "f16":
        return np.float16
    return np.float32


def build_bass(dtype=DTYPE):
    dt = _DT[dtype]
    nc = bacc.Bacc("TRN2", target_bir_lowering=False, debug=False,
                   num_devices=N_CORES)
    xd = nc.dram_tensor("xd", [128, 16, TW], dt, kind="ExternalInput")
    wr = nc.dram_tensor("wr", [128, TOT_COLS], dt, kind="ExternalInput")
    # consts: [8, 1152] = brT banks 0..3 (4x256) | lhsT0 (128)
    consts = nc.dram_tensor("consts", [8, NBANK * BANKC + 128], dt,
                            kind="ExternalInput")
    out = nc.dram_tensor("out", [128, NBANK * BANKC], dt,
                         kind="ExternalOutput")

    with tile.TileContext(nc) as tc:
        with (
            tc.tile_pool(name="cpool", bufs=1) as cpool,
            tc.tile_pool(name="xpool", bufs=1) as xpool,
            tc.tile_pool(name="wpool", bufs=NDMA) as wpool,
            tc.tile_pool(name="opool", bufs=2) as opool,
            tc.tile_pool(name="psum", bufs=2, space="PSUM") as pspool,
        ):
            # x tile: zero the off-diagonal blocks, DMA the diagonal ones
            x_sb = xpool.tile([128, 32, TW], dt)
            nc.vector.memset(x_sb[0:64, 16:32, :], 0.0)
            nc.vector.memset(x_sb[64:128, 0:16, :], 0.0)
            c_sb = cpool.tile([8, NBANK * BANKC + 128], dt)
            nc.scalar.dma_start(out=c_sb[:, :], in_=consts.ap())
            nc.sync.dma_start(out=x_sb[0:64, 0:16, :], in_=xd.ap()[0:64])
            nc.sync.dma_start(out=x_sb[64:128, 16:32, :], in_=xd.ap()[64:128])
            # weight chunks interleave 3 queues (2 HWDGE rings + SWDGE) in
            # consumption order so no queue idles while another backlogs
            w_engs = [nc.sync, nc.scalar, nc.gpsimd,
                      nc.sync, nc.scalar, nc.gpsimd,
                      nc.sync, nc.scalar]
            w_t = []
            for d in range(NDMA):
                wt = wpool.tile([128, CCPD * CHUNK_COLS], dt, tag="wt")
                w_engs[d].dma_start(
                    out=wt[:, :],
                    in_=wr.ap()[:, d * CCPD * CHUNK_COLS:
                                (d + 1) * CCPD * CHUNK_COLS])
                w_t.append(wt)

            for q in range(NBANK):
                ps = pspool.tile([128, BANKC], mybir.dt.float32, tag="ps")
                # bias pre-add: K=8 indicator matmul writes the whole bank
                nc.tensor.matmul(
                    ps[:, :],
                    lhsT=c_sb[:, NBANK * BANKC:NBANK * BANKC + 128],
                    rhs=c_sb[:, q * BANKC:(q + 1) * BANKC],
                    start=True, stop=False,
                )
                for g in range(NGRP):
                    cc = NGRP * q + g
                    base = cc * CHUNK_COLS - (cc // CCPD) * CCPD * CHUNK_COLS
                    wd = w_t[cc // CCPD]
                    for jj in range(NWIN):
                        t_abs = PPC * cc + jj
                        lo = max(PPC * cc, t_abs - (K - 1))
                        hi = min(PPC * cc + PPC - 1, t_abs)
                        nc.tensor.matmul(
                            ps[32 * g:32 * (g + 1),
                               (lo - PPC * cc) * WCOLS:
                               (hi + 1 - PPC * cc) * WCOLS],
                            lhsT=x_sb[:, :, t_abs],
                            rhs=wd[:, base + _OFFS[jj] * WCOLS:
                                   base + _OFFS[jj + 1] * WCOLS],
                            start=False,
                            stop=(g == NGRP - 1 and jj == NWIN - 1),
                            tile_position=(0, 32 * g),
                        )
                o_sb = opool.tile([128, BANKC], dt, tag="ot")
                nc.vector.tensor_copy(out=o_sb[:, :], in_=ps[:, :])
                eng = nc.scalar if q % 2 == 0 else nc.sync
                eng.dma_start(out=out.ap()[:, q * BANKC:(q + 1) * BANKC],
                              in_=o_sb[:, :])
    nc.compile()
    return nc


def _col_index_arrays():
    """Per-column (pair, tap, cout) indices for the consumption-order
    weight layout; identical for every core."""
    p_idx = np.empty(TOT_COLS, np.int64)
    k_idx = np.empty(TOT_COLS, np.int64)
    o_idx = np.empty(TOT_COLS, np.int64)
    col = 0
    for cc in range(NCC):
        for jj in range(NWIN):
            t_abs = PPC * cc + jj
            lo = max(PPC * cc, t_abs - (K - 1))
            hi = min(PPC * cc + PPC - 1, t_abs)
            for p in range(lo, hi + 1):
                p_idx[col:col + WCOLS] = p
                k_idx[col:col + WCOLS] = t_abs - p
                o_idx[col:col + WCOLS] = np.arange(WCOLS)
                col += WCOLS
    assert col == TOT_COLS
    return p_idx, k_idx, o_idx


_COL_IDX = None


def prep_inputs(input, weight, bias, dtype=DTYPE):
    """Host-side shard + relayout. Returns list of per-core input dicts."""
    global _COL_IDX
    if _COL_IDX is None:
        _COL_IDX = _col_index_arrays()
    p_idx, k_idx, o_idx = _COL_IDX
    npdt = _np_dt(dtype)

    xpad = np.pad(np.asarray(input, np.float32), ((0, 0), (0, 0), (1, 1)))
    wt = np.ascontiguousarray(
        np.asarray(weight, np.float32).transpose(1, 2, 3, 0))  # (cin,s,k,o)
    bias = np.asarray(bias, np.float32)

    # indicator lhsT0 [8, 128]: row r=2g+h hits partitions m=(g,h,b)
    l0 = np.zeros((8, 128), np.float32)
    m = np.arange(128)
    l0[2 * (m >> 5) + ((m >> 4) & 1), m] = 1.0

    in_maps = []
    for i in range(N_CORES):
        s0 = i * SC
        # x dense: [128, 16, TW]; partitions 0:64 half A, 64:128 half B
        xdn = np.empty((128, 16, TW), np.float32)
        xdn[:64] = xpad[:, :, s0:s0 + TW].transpose(1, 0, 2)
        xdn[64:] = xpad[:, :, s0 + H:s0 + H + TW].transpose(1, 0, 2)
        # w: [128, TOT_COLS] consumption order
        wr = np.empty((128, TOT_COLS), np.float32)
        wr[:64] = wt[:, s0 + p_idx, k_idx, o_idx]
        wr[64:] = wt[:, s0 + H + p_idx, k_idx, o_idx]
        # consts: brT banks + indicator
        cst = np.empty((8, NBANK * BANKC + 128), np.float32)
        b2 = bias[:, s0:s0 + SC]                 # (COUT, SC)
        for q in range(NBANK):
            for g in range(NGRP):
                for h in range(2):
                    base = 64 * h + 16 * q + 4 * g
                    cst[2 * g + h, q * BANKC:(q + 1) * BANKC] = (
                        b2[:, base:base + PPC].T.reshape(-1))
        cst[:, NBANK * BANKC:] = l0
        in_maps.append({
            "xd": np.ascontiguousarray(xdn.astype(npdt)),
            "wr": np.ascontiguousarray(wr.astype(npdt)),
            "consts": np.ascontiguousarray(cst.astype(npdt)),
        })
    return in_maps


def assemble_output(results):
    full = np.empty((B, COUT, S), np.float32)
    for i, r in enumerate(results):
        s0 = i * SC
        oc = np.asarray(r["out"], np.float32)    # (128, NBANK*BANKC)
        o6 = oc.reshape(NGRP, 2, 16, NBANK, PPC, WCOLS)   # g h b q pl o
        # s_local = 64h + 16q + 4g + pl
        full[:, :, s0:s0 + SC] = o6.transpose(2, 5, 1, 3, 0, 4).reshape(
            B, COUT, SC)
    return full


_CACHED = {}


def run(inputs, dtype=DTYPE, trace=False):
    if dtype not in _CACHED:
        _CACHED[dtype] = build_bass(dtype)
    nc = _CACHED[dtype]
    in_maps = prep_inputs(inputs["input"], inputs["weight"], inputs["bias"],
                          dtype)
    res = bass_utils.run_bass_kernel_spmd(
        nc, in_maps, core_ids=list(range(N_CORES)), trace=trace)
    return assemble_output(res.results), res


def kernel(input, weight, bias):
    out, _ = run({"input": input, "weight": weight, "bias": bias},
                 trace=False)
    return out
